# revision 13
# baseline (speedup 1.0000x reference)
"""Trainium2 Bass kernel for nn_CPCircuitLayer.

Math: with all_indices the full cartesian grid (s = n // H, h = n % H),
    out[b, s, h] = sum_r seq_emb[b,s,r] * hid_emb[b,h,r] * cp[r]
                 = (seq_emb[b] @ diag(cp) @ hid_emb[b].T)[s, h]
where seq_emb[b] = X_b @ seq_W.T  (X_b = hidden_states[b], contract H)
      hid_emb[b] = X_b.T @ hid_W.T                        (contract S)

Sharding: 8 cores = (batch b, seq half) pairs. Each core receives X_b
fully (the hid factor contracts over all of S) with rows rotated so its
own seq half comes first, plus a host-transposed copy of that half
(xt = X_b[half].T), and computes
    hid_embT = (hid_W*cp) @ X_b          [R, H]
    seq_embT = seq_W @ X_b[half].T       [R, S/2]
    out_half = seq_embT.T @ hid_embT     [S/2, H]
writing its [512, 1024] slice of the output.

Empirical HW model (from NTFF traces):
 - Two HWDGE queues (sync/Q1, scalar/Q10), each ~55-60 4KB-rows/us when
   both active; sync's first packet lands ~8.6us, scalar's ~10.9us.
   A transfer's completion semaphore = last of 16 striped engines, so
   completion lags first delivery by the full transfer drain time.
 - PE matmul cost ~= out free-size cols (512 cols ~= 215ns boosted,
   ~2x under active input DMA), independent of K.
 - The NEFF epilogue (semaphore scrub) is a fixed ~7.9us in the scored
   window; the startup barrier (~6.4us) is excluded.

Schedule: the hid factor and the out product are split by output
h-half (n0 = h 0:512, n1 = h 512:1024). X ships as column-half tiles
packed four per 4KB row, ordered so all n0 halves (plus the seq/xt
stream) land first: hid-n0 finishes mid-input, and the out-n0 matmuls,
PSUM evacuations and the n0 output DMA all overlap the n1 input
stream. Only the n1 half-chain remains as the tail. Out goes as two
128-row 4KB-row transfers, n0 on the scalar queue, n1 on sync.
"""

import numpy as np

B, S, H, R = 4, 1024, 1024, 32
N_CORES = 8
SH = S // 2    # seq rows per core
MT = SH // 128  # out row tiles (4)

# a_d column map (fp16 cols):
#  A1 [sw 256 | xt0 512 | hw 256 | x_k0n0 512 | x_k0n1 512]      0:2048
#  A2 [k1n0 | k2n0 | k3n0 | k4n0]                             2048:4096
#  A3 [k5n0 | k6n0 | k7n0 | k2n1]                             4096:6144
#  A4 [k3n1 | k4n1]                                           6144:7168
A_SW, A_XT0, A_HW, A_K0 = 0, 256, 768, 1024
A_COLS = 7168
# b_d column map:
#  B1 [xt1 | xt2 | xt3 | xt4]                                    0:2048
#  B2 [xt5 | xt6 | xt7 | k1n1]                                2048:4096
#  B3 [k5n1 | k6n1 | k7n1]                                    4096:5632
B_COLS = 5632

_compiled = {}


def _np_fallback(hidden_states, all_indices, seq_W, hid_W, cp_weight):
    seq_emb = np.einsum("bsh,rh->bsr", hidden_states, seq_W)
    hid_emb = np.einsum("bsh,rs->bhr", hidden_states, hid_W)
    s_idx = all_indices[:, 0].astype(np.int64)
    h_idx = all_indices[:, 1].astype(np.int64)
    g_seq = seq_emb[:, s_idx, :]
    g_hid = hid_emb[:, h_idx, :]
    out = np.einsum("bnr,bnr,r->bn", g_seq, g_hid, cp_weight[0])
    return out.reshape(B, S, H).astype(np.float32)


def _tile128(a):
    """[K*128, N] -> [128, K*N] with k-tiles adjacent in the free dim."""
    k = a.shape[0] // 128
    return np.ascontiguousarray(
        a.reshape(k, 128, a.shape[1]).transpose(1, 0, 2).reshape(128, -1))


def _wtile(w):
    """[K*128, R] -> [128, K*R] tile layout, partition-contiguous."""
    kt = w.shape[0] // 128
    return np.ascontiguousarray(
        w.reshape(kt, 128, R).transpose(1, 0, 2).reshape(128, kt * R))


def build_raw_program():
    import contextlib

    import concourse.bass as bass
    import concourse.mybir as mybir

    f32 = mybir.dt.float32
    f16 = mybir.dt.float16

    nc = bass.Bass("TRN2", target_bir_lowering=False, debug=False,
                   num_devices=N_CORES, enable_partition_id=False)

    a_d = nc.dram_tensor("a", [128, A_COLS], f16, kind="ExternalInput")
    b_d = nc.dram_tensor("b", [128, B_COLS], f16, kind="ExternalInput")
    out_d = nc.dram_tensor("out", [128, MT * H], f16, kind="ExternalOutput")

    with contextlib.ExitStack() as _xs:
        E = _xs.enter_context
        a_t = E(nc.sbuf_tensor([128, A_COLS], f16))
        b_t = E(nc.sbuf_tensor([128, B_COLS], f16))
        seq_sb = E(nc.sbuf_tensor([R, SH], f16))
        hid_sb = E(nc.sbuf_tensor([R, H], f16))
        # o_sb n-major: n half at cols n*2048, m-tile at +m*512
        o_sb = E(nc.sbuf_tensor([128, MT * H], f16))
        seq_ps = E(nc.psum_tensor([R, SH], f32))       # 1 bank
        hid_ps = E(nc.psum_tensor([R, H], f32))        # 2 banks
        o_ps = [E(nc.psum_tensor(f"o_ps{i}", [128, 512], f32))
                for i in range(5)]                     # 5 banks
        dma_sem = E(nc.semaphore("dma_sem"))
        pe_sem = E(nc.semaphore("pe_sem"))
        dve_sem = E(nc.semaphore("dve_sem"))
        act_sem = E(nc.semaphore("act_sem"))
        a_sem = [E(nc.semaphore(f"a_sem{j}")) for j in range(4)]
        b_sem = [E(nc.semaphore(f"b_sem{j}")) for j in range(3)]
        block = E(nc.Block(no_gpsimd_drain=True))

        ap_a = a_t.ap()
        ap_b = b_t.ap()
        sw = lambda k: ap_a[:, A_SW + k * R:A_SW + (k + 1) * R]
        hw = lambda k: ap_a[:, A_HW + k * R:A_HW + (k + 1) * R]

        def xt(k):
            if k == 0:
                return ap_a[:, A_XT0:A_XT0 + 512]
            if k <= 4:
                return ap_b[:, (k - 1) * 512:k * 512]
            return ap_b[:, 2048 + (k - 5) * 512:2048 + (k - 4) * 512]

        # column-half tiles x_k[n]: [128, 512]
        def xkn(k, n):
            if n == 0:
                if k == 0:
                    return ap_a[:, A_K0:A_K0 + 512]
                if k <= 4:
                    return ap_a[:, 2048 + (k - 1) * 512:2048 + k * 512]
                return ap_a[:, 4096 + (k - 5) * 512:4096 + (k - 4) * 512]
            if k == 0:
                return ap_a[:, A_K0 + 512:A_K0 + 1024]
            if k == 1:
                return ap_b[:, 3584:4096]
            if k == 2:
                return ap_a[:, 5632:6144]
            if k <= 4:
                return ap_a[:, 6144 + (k - 3) * 512:6144 + (k - 2) * 512]
            return ap_b[:, 4096 + (k - 5) * 512:4096 + (k - 4) * 512]

        # PE order (pe_sem counts):
        #   1 seq0 | 2 hid k0n0 | 3 hid k0n1 | 4-7 seq1..4
        #   8-11 hid k1..k4 n0 | 12-14 seq5..7 | 15 hid k1n1
        #   16-18 hid k5..k7 n0 (n0 stop) | 19 hid k2n1 | 20-22 hid k5..k7 n1
        #   23-24 hid k3,k4 n1 (n1 stop) | 25-28 out n0 | 29-32 out n1
        SEQ_DONE = 14
        HID_N0_DONE = 18
        HID_N1_DONE = 24
        # dve: seq=1, hidn0=2, n0m1=3, n0m3=4, n1m1=5, n1m3=6
        # act: hidn1=1, n0m0=2, n0m2=3, n1m0=4, n1m2=5
        N0_DVE, N0_ACT = 4, 3
        N1_DVE, N1_ACT = 6, 5

        @block.sync
        def _(sync):
            sync.dma_start(out=ap_a[:, 0:2048],
                           in_=a_d[:, 0:2048]).then_inc(a_sem[0], 16)
            sync.dma_start(out=ap_a[:, 2048:4096],
                           in_=a_d[:, 2048:4096]).then_inc(a_sem[1], 16)
            sync.dma_start(out=ap_a[:, 4096:6144],
                           in_=a_d[:, 4096:6144]).then_inc(a_sem[2], 16)
            sync.dma_start(out=ap_a[:, 6144:7168],
                           in_=a_d[:, 6144:7168]).then_inc(a_sem[3], 16)
            sync.wait_ge(dve_sem, N1_DVE)
            sync.wait_ge(act_sem, N1_ACT)
            sync.dma_start(out=out_d[:, 2048:4096],
                           in_=o_sb.ap()[:, 2048:4096]).then_inc(dma_sem, 16)
            sync.wait_ge(dma_sem, 32)

        @block.scalar
        def _(scalar):
            scalar.dma_start(out=ap_b[:, 0:2048],
                             in_=b_d[:, 0:2048]).then_inc(b_sem[0], 16)
            scalar.dma_start(out=ap_b[:, 2048:4096],
                             in_=b_d[:, 2048:4096]).then_inc(b_sem[1], 16)
            scalar.dma_start(out=ap_b[:, 4096:5632],
                             in_=b_d[:, 4096:5632]).then_inc(b_sem[2], 16)
            # dummy copy (garbage data) pulls the lazy ACT table load early
            nc.scalar.copy(o_sb.ap()[:, 0:R], o_sb.ap()[:, 1024:1024 + R])
            scalar.wait_ge(pe_sem, HID_N1_DONE)
            nc.scalar.copy(
                hid_sb.ap()[:, 512:1024],
                hid_ps.ap()[:, 512:1024]).then_inc(act_sem, 1)    # hid n1
            scalar.wait_ge(pe_sem, 25)
            nc.scalar.copy(o_sb.ap()[:, 0:512],
                           o_ps[0].ap()).then_inc(act_sem, 1)     # n0 m0
            scalar.wait_ge(pe_sem, 27)
            nc.scalar.copy(o_sb.ap()[:, 1024:1536],
                           o_ps[2].ap()).then_inc(act_sem, 1)     # n0 m2
            scalar.wait_ge(dve_sem, N0_DVE)
            scalar.dma_start(out=out_d[:, 0:2048],
                             in_=o_sb.ap()[:, 0:2048]).then_inc(dma_sem, 16)
            scalar.wait_ge(pe_sem, 29)
            nc.scalar.copy(o_sb.ap()[:, 2048:2560],
                           o_ps[4].ap()).then_inc(act_sem, 1)     # n1 m0
            scalar.wait_ge(pe_sem, 31)
            nc.scalar.copy(o_sb.ap()[:, 3072:3584],
                           o_ps[1].ap()).then_inc(act_sem, 1)     # n1 m2

        @block.tensor
        def _(tensor):
            def seq_k(k, start=False, stop=False):
                nc.tensor.matmul(
                    seq_ps.ap(), sw(k), xt(k), start=start, stop=stop,
                ).then_inc(pe_sem, 1)

            def hid_k(k, n, start=False, stop=False):
                nc.tensor.matmul(
                    hid_ps.ap()[:, n * 512:(n + 1) * 512],
                    hw(k), xkn(k, n), start=start, stop=stop,
                ).then_inc(pe_sem, 1)

            def out_mm(n, m, bank):
                nc.tensor.matmul(
                    o_ps[bank].ap(),
                    seq_sb.ap()[:, m * 128:(m + 1) * 128],
                    hid_sb.ap()[:, n * 512:(n + 1) * 512],
                    start=True, stop=True,
                ).then_inc(pe_sem, 1)

            tensor.wait_ge(a_sem[0], 16)
            seq_k(0, start=True)                       # pe 1
            hid_k(0, 0, start=True)                    # pe 2
            hid_k(0, 1, start=True)                    # pe 3
            tensor.wait_ge(b_sem[0], 16)
            for k in range(1, 5):
                seq_k(k)                               # pe 4-7
            tensor.wait_ge(a_sem[1], 16)
            for k in range(1, 5):
                hid_k(k, 0)                            # pe 8-11
            tensor.wait_ge(b_sem[1], 16)
            seq_k(5)
            seq_k(6)
            seq_k(7, stop=True)                        # pe 12-14
            hid_k(1, 1)                                # pe 15
            tensor.wait_ge(a_sem[2], 16)
            hid_k(5, 0)
            hid_k(6, 0)
            hid_k(7, 0, stop=True)                     # pe 16-18
            hid_k(2, 1)                                # pe 19
            tensor.wait_ge(b_sem[2], 16)
            hid_k(5, 1)
            hid_k(6, 1)
            hid_k(7, 1)                                # pe 20-22
            tensor.wait_ge(a_sem[3], 16)
            hid_k(3, 1)
            hid_k(4, 1, stop=True)                     # pe 23-24 (n1 stop)
            tensor.wait_ge(dve_sem, 2)   # seq + hid n0 copies done
            out_mm(0, 0, 0)                            # pe 25
            out_mm(0, 1, 1)                            # pe 26
            out_mm(0, 2, 2)                            # pe 27
            out_mm(0, 3, 3)                            # pe 28
            tensor.wait_ge(act_sem, 1)   # hid n1 copy done
            out_mm(1, 0, 4)                            # pe 29
            tensor.wait_ge(act_sem, 2)   # WAR bank0 (n0 m0 copied)
            out_mm(1, 1, 0)                            # pe 30
            tensor.wait_ge(dve_sem, 3)   # WAR bank1 (n0 m1 copied)
            out_mm(1, 2, 1)                            # pe 31
            tensor.wait_ge(act_sem, 3)   # WAR bank2 (n0 m2 copied)
            out_mm(1, 3, 2)                            # pe 32

        @block.vector
        def _(vector):
            vector.wait_ge(pe_sem, SEQ_DONE)
            nc.vector.tensor_copy(seq_sb.ap(), seq_ps.ap()).then_inc(
                dve_sem, 1)
            vector.wait_ge(pe_sem, HID_N0_DONE)
            nc.vector.tensor_copy(
                hid_sb.ap()[:, 0:512],
                hid_ps.ap()[:, 0:512]).then_inc(dve_sem, 1)       # hid n0
            for pe_at, dst, bank in ((26, 512, 1), (28, 1536, 3),
                                     (30, 2560, 0), (32, 3584, 2)):
                vector.wait_ge(pe_sem, pe_at)
                nc.vector.tensor_copy(
                    o_sb.ap()[:, dst:dst + 512],
                    o_ps[bank].ap()).then_inc(dve_sem, 1)

    return nc


def _get_program():
    if "nc" not in _compiled:
        _compiled["nc"] = build_raw_program()
    return _compiled["nc"]


def _make_in_maps(hidden_states, seq_W, hid_W, cp_weight):
    swT = _wtile(np.ascontiguousarray(seq_W.T, dtype=np.float16))  # [128, 256]
    hwT_rows = np.ascontiguousarray(
        (hid_W * cp_weight[0][:, None]).T, dtype=np.float16)       # [S, R]
    # per-half row rotation: own seq half first (hid contraction over S is
    # order-invariant as long as x rows and hw rows permute together)
    hw_rot = [
        _wtile(np.concatenate(
            [hwT_rows[half * SH:], hwT_rows[:half * SH]], axis=0))
        for half in range(2)
    ]
    in_maps = []
    for c in range(N_CORES):
        b, half = divmod(c, 2)
        xb = hidden_states[b].astype(np.float16)
        if half:
            xb = np.concatenate([xb[SH:], xb[:SH]], axis=0)
        xk = _tile128(xb)                                   # [128, 8192]
        xtk = _tile128(np.ascontiguousarray(xb[:SH, :].T))  # [128, 4096]
        hwt = hw_rot[half]                                  # [128, 256]

        def khalf(k, n):
            return xk[:, k * 1024 + n * 512:k * 1024 + (n + 1) * 512]

        a = np.empty((128, A_COLS), dtype=np.float16)
        a[:, A_SW:A_SW + 256] = swT
        a[:, A_XT0:A_XT0 + 512] = xtk[:, 0:512]
        a[:, A_HW:A_HW + 256] = hwt
        a[:, A_K0:A_K0 + 512] = khalf(0, 0)
        a[:, A_K0 + 512:A_K0 + 1024] = khalf(0, 1)
        for k in range(1, 5):
            a[:, 2048 + (k - 1) * 512:2048 + k * 512] = khalf(k, 0)
        for k in range(5, 8):
            a[:, 4096 + (k - 5) * 512:4096 + (k - 4) * 512] = khalf(k, 0)
        a[:, 5632:6144] = khalf(2, 1)
        a[:, 6144:6656] = khalf(3, 1)
        a[:, 6656:7168] = khalf(4, 1)

        bb = np.empty((128, B_COLS), dtype=np.float16)
        bb[:, 0:2048] = xtk[:, 512:2560]
        bb[:, 2048:3584] = xtk[:, 2560:4096]
        bb[:, 3584:4096] = khalf(1, 1)
        for k in range(5, 8):
            bb[:, 4096 + (k - 5) * 512:4096 + (k - 4) * 512] = khalf(k, 1)

        in_maps.append({"a": np.ascontiguousarray(a),
                        "b": np.ascontiguousarray(bb)})
    return in_maps


def kernel(hidden_states, all_indices, seq_W, hid_W, cp_weight):
    hidden_states = np.asarray(hidden_states, dtype=np.float32)
    seq_W = np.asarray(seq_W, dtype=np.float32)
    hid_W = np.asarray(hid_W, dtype=np.float32)
    cp_weight = np.asarray(cp_weight, dtype=np.float32)
    idx = np.asarray(all_indices)

    # The reference's all_indices is always the full cartesian grid; verify
    # cheaply and fall back to a host path if ever not.
    n = np.arange(S * H, dtype=idx.dtype)
    if idx.shape != (S * H, 2) or not (
        np.array_equal(idx[:, 0], n // H) and np.array_equal(idx[:, 1], n % H)
    ):
        return _np_fallback(hidden_states, idx, seq_W, hid_W, cp_weight)

    from concourse.bass_utils import run_bass_kernel_spmd

    nc = _get_program()
    in_maps = _make_in_maps(hidden_states, seq_W, hid_W, cp_weight)
    res = run_bass_kernel_spmd(nc, in_maps, list(range(N_CORES)))

    out = np.empty((B, S, H), dtype=np.float32)
    for c in range(N_CORES):
        b, half = divmod(c, 2)
        ot = res.results[c]["out"]  # [128, 4096] f16, n-major then m
        out[b, half * SH:(half + 1) * SH, :] = (
            ot.reshape(128, 2, MT, 512).transpose(2, 0, 1, 3)
            .reshape(SH, H).astype(np.float32))
    return out
